# revision 26
# baseline (speedup 1.0000x reference)
"""CRF loss kernel for Trainium2 (8 NeuronCores, data-parallel over batch).

Algorithm: the CRF forward recurrence fs_t[i] = LSE_j(sc[t,i,j] + fs_{t-1}[j])
runs in the exp domain as a positive matvec chain.  Scores live in SBUF
transposed per step: Esc[p=(q, j=prev tag), free=(t, g, i=cur tag)] with
example b_local = g*4 + q.  Each step is two DVE ops:

  tmp[(q,j),(g,i)] = Esc[t][(q,j),(g,i)] * v[(q,j), g]      (free-dim bcast)
  v'[(q,i), g]     = sum_j tmp[(q,j),(g,i)]                 (TRANSPOSE_TENSOR_REDUCE)

The TTR writes each step's state straight into the traj ring tile, which is
streamed to DRAM in slabs during the loop.  No explicit renormalization:
Esc = exp(sc - DRIFT) with DRIFT ~= the mean per-step log-growth (ln 32 +
ln E[e^sc] ~= 3.97), so the unnormalized chain stays centered near 1 with
+-O(sqrt(t)) log fluctuation, comfortably inside f32 range for S=512.  The
host takes log(traj[t*]) + DRIFT*(t*+1) at t* = len-1 per example (steps
past an example's length produce garbage that never flows backward).  The
gold score is a trivial masked gather done on the host during input prep.
"""

import numpy as np

B, S, T = 64, 512, 32
NCORES = 8
BPC = B // NCORES          # examples per core
QG, G = 4, 2               # partition-block examples, free-dim groups
DRIFT = 4.0                # per-step log-drift folded into exp(sc - DRIFT)
END = T - 1
GT = G * T

_CACHE = {}


def _build():
    import concourse.tile as tile
    from concourse import bacc, mybir

    f32 = mybir.dt.float32
    AF = mybir.ActivationFunctionType
    OP = mybir.AluOpType

    nc = bacc.Bacc("TRN2", target_bir_lowering=False, debug=False,
                   enable_asserts=True)

    sc = nc.dram_tensor("sc", [128, S * GT], f32, kind="ExternalInput").ap()
    traj = nc.dram_tensor("traj", [128, S * G], f32, kind="ExternalOutput").ap()

    def r3(ap):
        return ap.rearrange("p (g j) -> p g j", g=G)

    with tile.TileContext(nc) as tc:
        with (
            tc.tile_pool(name="big", bufs=1) as big_pool,
            tc.tile_pool(name="stage", bufs=3) as stage_pool,
            tc.tile_pool(name="state", bufs=4) as state_pool,
        ):
            Esc = big_pool.tile([128, S * GT], f32)
            nbias = big_pool.tile([128, 1], f32)
            nc.vector.memset(nbias[:], -DRIFT)
            # small first chunks so the scan chain starts ASAP
            bounds = [0, 8, 24, 56, 120]
            while bounds[-1] < S:
                bounds.append(min(bounds[-1] + 64, S))
            for c0, c1 in zip(bounds[:-1], bounds[1:]):
                stg = stage_pool.tile([128, (c1 - c0) * GT], f32, tag="stg")
                nc.sync.dma_start(stg[:], sc[:, c0 * GT:c1 * GT])
                # exp(sc - DRIFT): the constant bias keeps the unnormalized
                # chain inside f32 range for all 512 steps (growth/step
                # concentrates at ~3.94 nats); host adds DRIFT*(t*+1) back.
                nc.scalar.activation(Esc[:, c0 * GT:c1 * GT], stg[:], AF.Exp,
                                     bias=nbias[:])

            traj_t = big_pool.tile([128, S * G], f32)

            v0 = state_pool.tile([128, G], f32, tag="v")
            nc.vector.memset(v0[:], 1.0)
            v = v0[:]

            for t in range(S):
                tmp = state_pool.tile([128, GT], f32, tag="tmp")
                nc.vector.tensor_tensor(
                    r3(tmp[:]), r3(Esc[:, t * GT:(t + 1) * GT]),
                    v.unsqueeze(2).to_broadcast([128, G, T]), op=OP.mult)
                # raw (exp-domain) state written straight into the traj ring;
                # host takes log of row q*32+END at t*=len-1
                v2 = traj_t[:, t * G:(t + 1) * G]
                nc.vector.tensor_reduce(v2, r3(tmp[:]),
                                        axis=mybir.AxisListType.X,
                                        op=OP.add, apply_transpose=True)
                v = v2
                # stream finished traj slabs out while the loop runs
                if (t + 1) % 64 == 0 and t + 1 <= 448:
                    nc.sync.dma_start(traj[:, (t - 63) * G:(t + 1) * G],
                                      traj_t[:, (t - 63) * G:(t + 1) * G])
                elif t + 1 > 448 and (t + 1) % 16 == 0:
                    nc.sync.dma_start(traj[:, (t - 15) * G:(t + 1) * G],
                                      traj_t[:, (t - 15) * G:(t + 1) * G])

    nc.compile()
    return nc


def _prep_core_inputs(scores_core):
    """Host-side layout glue for one core's shard."""
    # device layout: sc[p=(q, j=prev), (t, g, i=cur)], example b_local = g*4+q
    dev = scores_core.reshape(G, QG, S, T, T)          # [g, q, t, i, j]
    dev = np.transpose(dev, (1, 4, 2, 0, 3))           # [q, j, t, g, i]
    sc_dev = np.ascontiguousarray(dev).reshape(128, S * GT).astype(np.float32)
    return {"sc": sc_dev}


def _gold_score(scores, targets, lengths):
    flat = scores.reshape(B, S, T * T)
    gathered = np.take_along_axis(
        flat, targets.astype(np.int64)[..., None], axis=2)[..., 0]  # [B,S]
    time_mask = np.arange(S)[None, :] < lengths[:, None]
    return float(np.sum(np.where(time_mask, gathered.astype(np.float64), 0.0)))


def _postprocess(results, lengths, gold_total):
    """Host-side gather of per-example answers + final sum."""
    total = 0.0
    for core in range(NCORES):
        traj = results[core]["traj"]                    # [128, S*G]
        for blc in range(BPC):
            b = core * BPC + blc
            q, g = blc % QG, blc // QG
            p = q * 32 + END
            tstar = int(lengths[b]) - 1
            total += (float(np.log(traj[p, tstar * G + g]))
                      + DRIFT * (tstar + 1))
    return np.float32(total - gold_total)


def kernel(scores, targets, lengths):
    from concourse import bass_utils

    scores = np.asarray(scores)
    targets = np.asarray(targets)
    lengths = np.asarray(lengths)

    if "nc" not in _CACHE:
        _CACHE["nc"] = _build()
    nc = _CACHE["nc"]

    in_maps = []
    for core in range(NCORES):
        sl = slice(core * BPC, (core + 1) * BPC)
        in_maps.append(_prep_core_inputs(scores[sl]))
    gold_total = _gold_score(scores, targets, lengths)

    res = bass_utils.run_bass_kernel_spmd(nc, in_maps,
                                          core_ids=list(range(NCORES)))
    _CACHE["last_results"] = res.results
    return _postprocess(res.results, lengths, gold_total)


# revision 28
# speedup vs baseline: 1.0038x; 1.0038x over previous
"""CRF loss kernel for Trainium2 (8 NeuronCores, data-parallel over batch).

Algorithm: the CRF forward recurrence fs_t[i] = LSE_j(sc[t,i,j] + fs_{t-1}[j])
runs in the exp domain as a positive matvec chain.  Scores live in SBUF
transposed per step: Esc[p=(q, j=prev tag), free=(t, g, i=cur tag)] with
example b_local = g*4 + q.  Each step is two DVE ops:

  tmp[(q,j),(g,i)] = Esc[t][(q,j),(g,i)] * v[(q,j), g]      (free-dim bcast)
  v'[(q,i), g]     = sum_j tmp[(q,j),(g,i)]                 (TRANSPOSE_TENSOR_REDUCE)

The TTR writes each step's state straight into the traj ring tile, which is
streamed to DRAM in slabs during the loop.  No explicit renormalization:
Esc = exp(sc - DRIFT) with DRIFT ~= the mean per-step log-growth (ln 32 +
ln E[e^sc] ~= 3.97), so the unnormalized chain stays centered near 1 with
+-O(sqrt(t)) log fluctuation, comfortably inside f32 range for S=512.  The
host takes log(traj[t*]) + DRIFT*(t*+1) at t* = len-1 per example (steps
past an example's length produce garbage that never flows backward).  The
gold score is a trivial masked gather done on the host during input prep.
"""

import numpy as np

B, S, T = 64, 512, 32
NCORES = 8
BPC = B // NCORES          # examples per core
QG, G = 4, 2               # partition-block examples, free-dim groups
DRIFT = 4.0                # per-step log-drift folded into exp(sc - DRIFT)
END = T - 1
GT = G * T

_CACHE = {}


def _build():
    import concourse.tile as tile
    from concourse import bacc, mybir

    f32 = mybir.dt.float32
    AF = mybir.ActivationFunctionType
    OP = mybir.AluOpType

    nc = bacc.Bacc("TRN2", target_bir_lowering=False, debug=False,
                   enable_asserts=True)

    sc = nc.dram_tensor("sc", [128, S * GT], f32, kind="ExternalInput").ap()
    traj = nc.dram_tensor("traj", [128, S * G], f32, kind="ExternalOutput").ap()

    def r3(ap):
        return ap.rearrange("p (g j) -> p g j", g=G)

    with tile.TileContext(nc) as tc:
        with (
            tc.tile_pool(name="big", bufs=1) as big_pool,
            tc.tile_pool(name="stage", bufs=3) as stage_pool,
            tc.tile_pool(name="state", bufs=4) as state_pool,
        ):
            Esc = big_pool.tile([128, S * GT], f32)
            nbias = big_pool.tile([128, 1], f32)
            nc.vector.memset(nbias[:], -DRIFT)
            # small first chunks so the scan chain starts ASAP
            bounds = [0, 4, 12, 28, 60, 124]
            while bounds[-1] < S:
                bounds.append(min(bounds[-1] + 64, S))
            for c0, c1 in zip(bounds[:-1], bounds[1:]):
                stg = stage_pool.tile([128, (c1 - c0) * GT], f32, tag="stg")
                nc.sync.dma_start(stg[:], sc[:, c0 * GT:c1 * GT])
                # exp(sc - DRIFT): the constant bias keeps the unnormalized
                # chain inside f32 range for all 512 steps (growth/step
                # concentrates at ~3.94 nats); host adds DRIFT*(t*+1) back.
                nc.scalar.activation(Esc[:, c0 * GT:c1 * GT], stg[:], AF.Exp,
                                     bias=nbias[:])

            traj_t = big_pool.tile([128, S * G], f32)

            v0 = state_pool.tile([128, G], f32, tag="v")
            nc.vector.memset(v0[:], 1.0)
            v = v0[:]

            for t in range(S):
                tmp = state_pool.tile([128, GT], f32, tag="tmp")
                nc.vector.tensor_tensor(
                    r3(tmp[:]), r3(Esc[:, t * GT:(t + 1) * GT]),
                    v.unsqueeze(2).to_broadcast([128, G, T]), op=OP.mult)
                # raw (exp-domain) state written straight into the traj ring;
                # host takes log of row q*32+END at t*=len-1
                v2 = traj_t[:, t * G:(t + 1) * G]
                nc.vector.tensor_reduce(v2, r3(tmp[:]),
                                        axis=mybir.AxisListType.X,
                                        op=OP.add, apply_transpose=True)
                v = v2
                # stream finished traj slabs out while the loop runs
                if (t + 1) % 64 == 0 and t + 1 <= 448:
                    nc.sync.dma_start(traj[:, (t - 63) * G:(t + 1) * G],
                                      traj_t[:, (t - 63) * G:(t + 1) * G])
                elif 448 < t + 1 <= 496 and (t + 1) % 16 == 0:
                    nc.sync.dma_start(traj[:, (t - 15) * G:(t + 1) * G],
                                      traj_t[:, (t - 15) * G:(t + 1) * G])
                elif t + 1 > 496 and (t + 1) % 8 == 0:
                    nc.sync.dma_start(traj[:, (t - 7) * G:(t + 1) * G],
                                      traj_t[:, (t - 7) * G:(t + 1) * G])

    nc.compile()
    return nc


def _prep_core_inputs(scores_core):
    """Host-side layout glue for one core's shard."""
    # device layout: sc[p=(q, j=prev), (t, g, i=cur)], example b_local = g*4+q
    dev = scores_core.reshape(G, QG, S, T, T)          # [g, q, t, i, j]
    dev = np.transpose(dev, (1, 4, 2, 0, 3))           # [q, j, t, g, i]
    sc_dev = np.ascontiguousarray(dev).reshape(128, S * GT).astype(np.float32)
    return {"sc": sc_dev}


def _gold_score(scores, targets, lengths):
    flat = scores.reshape(B, S, T * T)
    gathered = np.take_along_axis(
        flat, targets.astype(np.int64)[..., None], axis=2)[..., 0]  # [B,S]
    time_mask = np.arange(S)[None, :] < lengths[:, None]
    return float(np.sum(np.where(time_mask, gathered.astype(np.float64), 0.0)))


def _postprocess(results, lengths, gold_total):
    """Host-side gather of per-example answers + final sum."""
    total = 0.0
    for core in range(NCORES):
        traj = results[core]["traj"]                    # [128, S*G]
        for blc in range(BPC):
            b = core * BPC + blc
            q, g = blc % QG, blc // QG
            p = q * 32 + END
            tstar = int(lengths[b]) - 1
            total += (float(np.log(traj[p, tstar * G + g]))
                      + DRIFT * (tstar + 1))
    return np.float32(total - gold_total)


def kernel(scores, targets, lengths):
    from concourse import bass_utils

    scores = np.asarray(scores)
    targets = np.asarray(targets)
    lengths = np.asarray(lengths)

    if "nc" not in _CACHE:
        _CACHE["nc"] = _build()
    nc = _CACHE["nc"]

    in_maps = []
    for core in range(NCORES):
        sl = slice(core * BPC, (core + 1) * BPC)
        in_maps.append(_prep_core_inputs(scores[sl]))
    gold_total = _gold_score(scores, targets, lengths)

    res = bass_utils.run_bass_kernel_spmd(nc, in_maps,
                                          core_ids=list(range(NCORES)))
    _CACHE["last_results"] = res.results
    return _postprocess(res.results, lengths, gold_total)
